# revision 1
# baseline (speedup 1.0000x reference)
"""Trainium2 Bass kernel for nn_ContrastiveLoss (SimCLR-style, N=8192, D=128).

Sharding: rows of the NxN sim matrix split across 8 cores (1024 rows each).
Each core receives the full z = concat(emb0, emb1) ROTATED so its own rows
come first (np.roll(z, -core*1024, axis=0)).  With that rotation the diagonal
of row-block b sits at local columns [b*128, b*128+128) and the positive pair
at local columns [4096+b*128, ...), identical on every core -> one SPMD
program, no collectives.

Math (per row r, fixed max = 1.0 since cosine sim <= 1):
  e_j  = exp(10*G_rj - 10),  S_r = sum_j e_j - e_rr
  loss_r = lse_r - 10*G_pos = (10 + ln S_r) - (ln e_pos + 10) = ln S_r - ln e_pos
  loss   = mean_r(loss_r);  per-core output = [128,1] partial sums of loss_r.

Engine split per core: PE does z_blk @ z^T (bf16 operands, fp32 psum)
plus the zn transposes; ACT does exp(10x-10) on each [128,2048] psum chunk
with accum_out row-sums; DVE does norms, psum->bf16 casts (batched 512 wide)
and diag/pos extraction from the exp output in SBUF.
"""

import sys

sys.path.insert(0, "/opt/trn_rl_repo")

from contextlib import ExitStack

import numpy as np

import concourse.bass as bass
import concourse.bacc as bacc
import concourse.tile as tile
from concourse import mybir
from concourse import bass_utils
from concourse.masks import make_identity

B = 4096
D = 128
N = 2 * B            # 8192 rows of z
NCORES = 8
ROWS = N // NCORES   # 1024 rows per core
NBLK = ROWS // 128   # 8 row-blocks per core
CHUNK = 2048         # psum tile width (4 banks)
NCHUNK = N // CHUNK  # 4 column chunks
SEG = 512            # matmul moving-operand width
NTILE = N // 128     # 64 partition-tiles of z
GRP = 8              # tiles per DMA / norm group
INV_T = 10.0         # 1/temperature
EPS = 1e-8

F32 = mybir.dt.float32
BF16 = mybir.dt.bfloat16
AX = mybir.AxisListType
AF = mybir.ActivationFunctionType


def _build() -> bass.Bass:
    nc = bacc.Bacc(None)
    z_in = nc.declare_dram_parameter("z", [N, D], F32, isOutput=False)
    out = nc.declare_dram_parameter("partial", [128, 1], F32, isOutput=True)

    z_re = z_in.rearrange("(n p) d -> p n d", p=128)  # row = n*128 + p

    with tile.TileContext(nc) as tc:
        with ExitStack() as ctx:
            persist = ctx.enter_context(tc.tile_pool(name="persist", bufs=1))
            work = ctx.enter_context(tc.tile_pool(name="work", bufs=3))
            junkp = ctx.enter_context(tc.tile_pool(name="junk", bufs=3))
            psum = ctx.enter_context(tc.tile_pool(name="psum", bufs=2, space="PSUM"))

            ident = persist.tile([128, 128], BF16)
            make_identity(nc, ident)
            # non-Copy activations need bias as an SBUF AP
            b_zero = persist.tile([128, 1], F32)
            nc.vector.memset(b_zero, 0.0)
            b_neg10 = persist.tile([128, 1], F32)
            nc.vector.memset(b_neg10, -INV_T)

            # ---- load z + per-group norms + normalize + transpose --------
            # Per 8-tile group: DMA -> sumsq -> rsqrt-norm -> bf16 zn ->
            # 8 PE transposes into one psum bank -> one 1024-wide cast.
            z_sb = persist.tile([128, NTILE, D], F32)
            sq = persist.tile([128, NTILE, D], F32)
            rn = persist.tile([128, NTILE], F32)
            zn_all = persist.tile([128, NTILE, D], BF16)
            znT = [
                persist.tile([128, CHUNK], BF16, tag=f"znT{j}", name=f"znT{j}")
                for j in range(NCHUNK)
            ]
            acc = persist.tile([128, NBLK, NCHUNK], F32)   # per-chunk exp sums
            e_diag = persist.tile([128, NBLK], F32)
            e_pos = persist.tile([128, NBLK], F32)

            # all input DMAs up front; the sync queue streams them back-to-back
            for i in range(NTILE // GRP):
                sl = slice(i * GRP, (i + 1) * GRP)
                nc.sync.dma_start(out=z_sb[:, sl, :], in_=z_re[:, sl, :])

            def norm_group(i):
                sl = slice(i * GRP, (i + 1) * GRP)
                nc.vector.tensor_mul(sq[:, sl, :], z_sb[:, sl, :], z_sb[:, sl, :])
                nc.vector.reduce_sum(rn[:, sl], sq[:, sl, :], axis=AX.X)
                nc.scalar.activation(rn[:, sl], rn[:, sl], AF.Sqrt, bias=b_zero)
                nc.vector.tensor_scalar_max(rn[:, sl], rn[:, sl], EPS)
                nc.vector.reciprocal(rn[:, sl], rn[:, sl])
                nc.vector.tensor_mul(
                    zn_all[:, sl, :],
                    z_sb[:, sl, :],
                    rn[:, sl].broadcast_to((128, GRP, D)),
                )
                tp = psum.tile([128, GRP * 128], BF16, tag="pp", name="tp")
                for q in range(GRP):
                    nc.tensor.transpose(
                        tp[:, q * 128 : (q + 1) * 128],
                        zn_all[:, i * GRP + q, :],
                        ident,
                    )
                j, k = divmod(i * GRP * 128, CHUNK)
                nc.vector.tensor_copy(znT[j][:, k : k + GRP * 128], tp)

            def emit_block(b, c):
                lhsT = znT[0][:, b * 128 : (b + 1) * 128]  # block cols < 1024
                pt = psum.tile([128, CHUNK], F32, tag="pp", name="pt")
                for s in range(CHUNK // SEG):
                    nc.tensor.matmul(
                        pt[:, s * SEG : (s + 1) * SEG],
                        lhsT,
                        znT[c][:, s * SEG : (s + 1) * SEG],
                        start=True,
                        stop=True,
                    )
                ej = junkp.tile([128, CHUNK], F32, tag="ej", name="ej")
                nc.scalar.activation(
                    ej, pt, AF.Exp, scale=INV_T, bias=b_neg10,
                    accum_out=acc[:, b, c : c + 1],
                )
                if c == 0:  # e_rr at cols b*128..+128 of chunk 0
                    scr = work.tile([128, 128], F32, tag="scr", name="scr")
                    nc.vector.tensor_mul(scr, ej[:, b * 128 : b * 128 + 128], ident)
                    nc.vector.reduce_sum(e_diag[:, b : b + 1], scr, axis=AX.X)
                if c == 2:  # e_pos at cols 4096 + b*128..+128
                    scr2 = work.tile([128, 128], F32, tag="scr2", name="scr2")
                    nc.vector.tensor_mul(scr2, ej[:, b * 128 : b * 128 + 128], ident)
                    nc.vector.reduce_sum(e_pos[:, b : b + 1], scr2, axis=AX.X)

            # Pass 0 interleaves the remaining norm groups PAIRWISE so the
            # 2-slot psum round-robin keeps consecutive pt tiles on opposite
            # slots (tp pairs between pt pairs); all znT chunks are ready
            # before pass 1 and the exp stream starts as soon as znT[0] is.
            norm_group(0); norm_group(1)
            emit_block(0, 0); emit_block(1, 0)
            norm_group(2); norm_group(3)
            emit_block(2, 0); emit_block(3, 0)
            norm_group(4); norm_group(5)
            emit_block(4, 0); emit_block(5, 0)
            norm_group(6); norm_group(7)
            emit_block(6, 0); emit_block(7, 0)
            for c in range(1, NCHUNK):
                for b in range(NBLK):
                    emit_block(b, c)

            # ---- epilogue ------------------------------------------------
            sumexp = persist.tile([128, NBLK], F32)
            nc.vector.reduce_sum(sumexp, acc, axis=AX.X)      # [128,8,4] -> [128,8]
            S = persist.tile([128, NBLK], F32)
            nc.vector.tensor_sub(S, sumexp, e_diag)
            lnS = persist.tile([128, NBLK], F32)
            nc.scalar.activation(lnS, S, AF.Ln, bias=b_zero)
            lnp = persist.tile([128, NBLK], F32)
            nc.scalar.activation(lnp, e_pos, AF.Ln, bias=b_zero)
            contrib = persist.tile([128, NBLK], F32)
            nc.vector.tensor_sub(contrib, lnS, lnp)
            total = persist.tile([128, 1], F32)
            nc.vector.reduce_sum(total, contrib, axis=AX.X)
            nc.sync.dma_start(out=out[:, :], in_=total)

    nc.compile()
    return nc


_NC = None


def _get_nc() -> bass.Bass:
    global _NC
    if _NC is None:
        _NC = _build()
    return _NC


def kernel(emb0: np.ndarray, emb1: np.ndarray) -> np.ndarray:
    z = np.concatenate(
        [np.asarray(emb0, np.float32), np.asarray(emb1, np.float32)], axis=0
    )
    in_maps = [
        {"z": np.ascontiguousarray(np.roll(z, -c * ROWS, axis=0))}
        for c in range(NCORES)
    ]
    res = bass_utils.run_bass_kernel_spmd(_get_nc(), in_maps, core_ids=list(range(NCORES)))
    total = sum(float(r["partial"].sum(dtype=np.float64)) for r in res.results)
    return np.asarray(np.float32(total / N))



# revision 2
# speedup vs baseline: 1.2130x; 1.2130x over previous
"""Trainium2 Bass kernel for nn_ContrastiveLoss (SimCLR-style, N=8192, D=128).

v2: pair-symmetric sharding.  sim is symmetric, so each unordered pair
{i,j} only needs ONE exp.  Core c receives z rotated by -c*1024 rows and
TRUNCATED to its first 5120 rows: its own 1024 rows (the row block) plus
the next 4096 rows (column groups c+1..c+4 mod 8).  It computes
e = exp(10*cos - 10) for rows x all 5120 local columns:

  - row sums (ACT accum_out)          -> partial S for its OWN 1024 rows
  - column sums over local cols 1024..4095 (PE ones-matmul) -> partial S
    for rows owned by cores c+1..c+3 (those cores never look back at
    block c).  Cols 0..1023 need no colsum (own diag block is computed
    in full, so both (r,s) and (s,r) land in local row sums).  Cols
    4096..5119 need no colsum either: the {c, c+4} block is computed by
    BOTH endpoints (each keeps only its row sums).
  - Sum_r pos-cosine via a DVE dot of zn rows 0..1023 with rows
    4096..5119 (the positive pairs); no exp needed since
    ln e_pos = 10*g_pos - 10 exactly.

Every global pair lands in exactly one partial sum of each of its two
rows.  Host assembles S_r = rowsum + colsums - 1 (diag, e_rr = 1), then
loss = mean(ln S_r + 10 - 10 g_pos_r).  exp work per core drops from
64 to 40 col-tiles (0.625x) - ACT at 1 elem/cycle/lane is the hard
bottleneck engine for this kernel.

Other changes vs v1: 1/||z|| computed as exp(-0.5*ln(ss)) so the whole
kernel uses ONE activation table set (natural_log_exp); no diag/pos
extraction from e; e stored bf16 (halves ACT SBUF writes, feeds PE
colsum matmuls).
"""

import sys

sys.path.insert(0, "/opt/trn_rl_repo")

from contextlib import ExitStack

import numpy as np

import concourse.bass as bass
import concourse.bacc as bacc
import concourse.tile as tile
from concourse import mybir
from concourse import bass_utils
from concourse.masks import make_identity

B = 4096
D = 128
N = 2 * B            # 8192 rows of z
NCORES = 8
ROWS = N // NCORES   # 1024 rows per core
NBLK = ROWS // 128   # 8 row-blocks per core
NT = 40              # local column tiles kept per core (5 block-columns)
COLS = NT * 128      # 5120 local columns
GRP = 8              # tiles per DMA / norm group
NGRP = NT // GRP     # 5 groups
CS_LO = 1024         # colsummed local columns [CS_LO, CS_HI)
CS_HI = 4096
INV_T = 10.0         # 1/temperature

F32 = mybir.dt.float32
BF16 = mybir.dt.bfloat16
AX = mybir.AxisListType
AF = mybir.ActivationFunctionType

# column chunks of the exp stream: (base, width, psum banks)
CHUNKS = [(0, 2048), (2048, 2048), (4096, 1024)]


def _build() -> bass.Bass:
    nc = bacc.Bacc(None)
    z_in = nc.declare_dram_parameter("z", [COLS, D], F32, isOutput=False)
    out_row = nc.declare_dram_parameter("rowsum", [128, NBLK], F32, isOutput=True)
    out_cs = nc.declare_dram_parameter("colsum", [1, CS_HI - CS_LO], F32, isOutput=True)
    out_pos = nc.declare_dram_parameter("possum", [128, 1], F32, isOutput=True)

    z_re = z_in.rearrange("(n p) d -> p n d", p=128)  # local row = n*128 + p

    with tile.TileContext(nc) as tc:
        with ExitStack() as ctx:
            persist = ctx.enter_context(tc.tile_pool(name="persist", bufs=1))
            work = ctx.enter_context(tc.tile_pool(name="work", bufs=2))
            psum = ctx.enter_context(tc.tile_pool(name="psum", bufs=2, space="PSUM"))

            ident = persist.tile([128, 128], BF16)
            make_identity(nc, ident)
            ones_col = persist.tile([128, 1], BF16)
            nc.vector.memset(ones_col, 1.0)
            b_zero = persist.tile([128, 1], F32)
            nc.vector.memset(b_zero, 0.0)
            b_neg10 = persist.tile([128, 1], F32)
            nc.vector.memset(b_neg10, -INV_T)

            z_sb = persist.tile([128, NT, D], F32)
            zb = persist.tile([128, NT, D], BF16)
            sq = persist.tile([128, NT, D], BF16)
            ss = persist.tile([128, NT], F32)
            rn = persist.tile([128, NT], F32)
            zn = persist.tile([128, NT, D], BF16)
            znT = persist.tile([128, COLS], BF16)
            ej = persist.tile([128, NBLK, COLS], BF16)
            acc = persist.tile([128, NBLK, len(CHUNKS)], F32)
            cs_sb = persist.tile([1, CS_HI - CS_LO], F32)

            for g in range(NGRP):
                sl = slice(g * GRP, (g + 1) * GRP)
                nc.sync.dma_start(out=z_sb[:, sl, :], in_=z_re[:, sl, :])

            def norm_group(g):
                sl = slice(g * GRP, (g + 1) * GRP)
                nc.vector.tensor_copy(zb[:, sl, :], z_sb[:, sl, :])
                nc.vector.tensor_mul(sq[:, sl, :], zb[:, sl, :], zb[:, sl, :])
                nc.vector.reduce_sum(ss[:, sl], sq[:, sl, :], axis=AX.X)
                # 1/||z|| = exp(-0.5 ln ss): stays in the natural_log_exp
                # table set (no ACT table switch for sqrt)
                nc.scalar.activation(rn[:, sl], ss[:, sl], AF.Ln, bias=b_zero)
                nc.scalar.activation(rn[:, sl], rn[:, sl], AF.Exp, scale=-0.5, bias=b_zero)
                nc.vector.tensor_mul(
                    zn[:, sl, :],
                    zb[:, sl, :],
                    rn[:, sl].broadcast_to((128, GRP, D)),
                )
                tp = psum.tile([128, GRP * 128], BF16, tag="pp", name="tp")
                for q in range(GRP):
                    nc.tensor.transpose(
                        tp[:, q * 128 : (q + 1) * 128],
                        zn[:, g * GRP + q, :],
                        ident,
                    )
                nc.vector.tensor_copy(znT[:, g * 1024 : (g + 1) * 1024], tp)

            for g in range(NGRP):
                norm_group(g)

            def emit_chunk(ci, b):
                base, cw = CHUNKS[ci]
                pt = psum.tile([128, 2048], F32, tag="pp", name="pt")
                for s in range(cw // 512):
                    nc.tensor.matmul(
                        pt[:, s * 512 : (s + 1) * 512],
                        znT[:, b * 128 : (b + 1) * 128],
                        znT[:, base + s * 512 : base + (s + 1) * 512],
                        start=True,
                        stop=True,
                    )
                nc.scalar.activation(
                    ej[:, b, base : base + cw],
                    pt[:, :cw],
                    AF.Exp,
                    scale=INV_T,
                    bias=b_neg10,
                    accum_out=acc[:, b, ci : ci + 1],
                )

            def emit_colsums(ci):
                base, cw = CHUNKS[ci]
                for off in range(max(base, CS_LO), min(base + cw, CS_HI), 512):
                    cs = psum.tile([1, 512], F32, tag="pp", name="cs")
                    for b in range(NBLK):
                        nc.tensor.matmul(
                            cs,
                            ones_col,
                            ej[:, b, off : off + 512],
                            start=(b == 0),
                            stop=(b == NBLK - 1),
                        )
                    nc.vector.tensor_copy(
                        cs_sb[:, off - CS_LO : off - CS_LO + 512], cs
                    )

            # colsums for chunk c are emitted after chunk c+1's matmuls so
            # the in-order PE queue never stalls waiting for ACT to finish
            # chunk c's exps.
            for b in range(NBLK):
                emit_chunk(0, b)
            for b in range(NBLK):
                emit_chunk(1, b)
            emit_colsums(0)
            for b in range(NBLK):
                emit_chunk(2, b)
            emit_colsums(1)

            # ---- epilogue ------------------------------------------------
            rowsum = persist.tile([128, NBLK], F32)
            nc.vector.reduce_sum(rowsum, acc, axis=AX.X)
            # sum_r pos-cosine: zn rows 0..1023 dot zn rows 4096..5119
            pm = work.tile([128, NBLK, D], BF16, tag="pm", name="pm")
            nc.vector.tensor_mul(pm, zn[:, 0:NBLK, :], zn[:, 4 * NBLK : 5 * NBLK, :])
            pr = work.tile([128, NBLK], F32, tag="pr", name="pr")
            nc.vector.reduce_sum(pr, pm, axis=AX.X)
            possum = persist.tile([128, 1], F32)
            nc.vector.reduce_sum(possum, pr, axis=AX.X)

            nc.sync.dma_start(out=out_row[:, :], in_=rowsum)
            nc.sync.dma_start(out=out_cs[:, :], in_=cs_sb)
            nc.sync.dma_start(out=out_pos[:, :], in_=possum)

    nc.compile()
    return nc


_NC = None


def _get_nc() -> bass.Bass:
    global _NC
    if _NC is None:
        _NC = _build()
    return _NC


def make_in_maps(z: np.ndarray) -> list[dict]:
    return [
        {"z": np.ascontiguousarray(np.roll(z, -c * ROWS, axis=0)[:COLS])}
        for c in range(NCORES)
    ]


def kernel(emb0: np.ndarray, emb1: np.ndarray) -> np.ndarray:
    z = np.concatenate(
        [np.asarray(emb0, np.float32), np.asarray(emb1, np.float32)], axis=0
    )
    res = bass_utils.run_bass_kernel_spmd(
        _get_nc(), make_in_maps(z), core_ids=list(range(NCORES))
    )
    # assemble full row sums of exp(10 cos - 10) from per-core partials
    S = np.zeros(N, dtype=np.float64)
    pos_total = 0.0
    for c, r in enumerate(res.results):
        rows = r["rowsum"].astype(np.float64).T.reshape(ROWS)  # local row n*128+p
        S[c * ROWS : (c + 1) * ROWS] += rows
        idx = (c * ROWS + CS_LO + np.arange(CS_HI - CS_LO)) % N
        np.add.at(S, idx, r["colsum"].astype(np.float64).reshape(-1))
        pos_total += float(r["possum"].sum(dtype=np.float64))
    S -= 1.0  # remove the diagonal term exp(10*1 - 10) = 1
    total = float(np.sum(np.log(S))) + INV_T * N - INV_T * pos_total
    return np.asarray(np.float32(total / N))


# revision 6
# speedup vs baseline: 1.2169x; 1.0032x over previous
"""Trainium2 Bass kernel for nn_ContrastiveLoss (SimCLR-style, N=8192, D=128).

v3: pair-symmetric sharding (see v2 notes below) + DVE-only norm path.

Sharding: sim is symmetric, so each unordered pair {i,j} needs ONE exp.
Core c gets z rotated by -c*1024 rows, truncated to 5120 rows: its own
1024 rows plus the next 4 block-columns.  It computes
e = exp(10*cos - 10) for rows x all 5120 local columns:
  - row sums (ACT accum_out)      -> partial S for its OWN rows
  - col sums over local cols 1024..4095 (PE ones-matmul) -> partial S
    for rows owned by cores c+1..c+3.  Cols 0..1023 (own diag block,
    computed in full) and 4096..5119 (the {c,c+4} block, computed by
    BOTH endpoints, row sums only) need no colsum.
  - sum_r pos-cosine via a DVE dot (ln e_pos = 10 g_pos - 10 exactly).
Host assembles S_r = rowsum + colsums - 1 (diag), then
loss = mean(ln S_r + 10 - 10 g_pos_r).  exp work per core is 0.625x of
the full-matrix version; ACT (1 elem/cycle/lane @1.2GHz) is the
bottleneck engine.

v3 changes: 1/||z|| = bit-trick rsqrt + 2 Newton steps entirely on DVE
(no ACT sqrt/ln -> ONE act table load, no DVE<->ACT ping-pong in the
prologue); a dummy exp primes the table during the input DMA; emission
order lets chunk-0 exps start after only groups 0-1 are normalized,
with groups 2-4 norming under the chunk-0 exp stream.
"""

import sys

sys.path.insert(0, "/opt/trn_rl_repo")

from contextlib import ExitStack

import numpy as np

import concourse.bass as bass
import concourse.bacc as bacc
import concourse.tile as tile
from concourse import mybir
from concourse import bass_utils
from concourse.masks import make_identity

B = 4096
D = 128
N = 2 * B            # 8192 rows of z
NCORES = 8
ROWS = N // NCORES   # 1024 rows per core
NBLK = ROWS // 128   # 8 row-blocks per core
NT = 40              # local column tiles kept per core (5 block-columns)
COLS = NT * 128      # 5120 local columns
GRP = 8              # tiles per DMA / norm group
NGRP = NT // GRP     # 5 groups
CS_LO = 1024         # colsummed local columns [CS_LO, CS_HI)
CS_HI = 4096
INV_T = 10.0         # 1/temperature
MAGIC = 0x5F3759DF   # fp32 rsqrt seed

F32 = mybir.dt.float32
BF16 = mybir.dt.bfloat16
I32 = mybir.dt.int32
AX = mybir.AxisListType
AF = mybir.ActivationFunctionType
OP = mybir.AluOpType

# column chunks of the exp stream: (base, width)
CHUNKS = [(0, 2048), (2048, 2048), (4096, 1024)]


def _build() -> bass.Bass:
    nc = bacc.Bacc(None)
    z_in = nc.declare_dram_parameter("z", [COLS, D], F32, isOutput=False)
    out_row = nc.declare_dram_parameter("rowsum", [128, NBLK], F32, isOutput=True)
    out_cs = nc.declare_dram_parameter("colsum", [1, CS_HI - CS_LO], F32, isOutput=True)
    out_pos = nc.declare_dram_parameter("possum", [128, 1], F32, isOutput=True)

    z_re = z_in.rearrange("(n p) d -> p n d", p=128)  # local row = n*128 + p

    with tile.TileContext(nc) as tc:
        with ExitStack() as ctx:
            persist = ctx.enter_context(tc.tile_pool(name="persist", bufs=1))
            work = ctx.enter_context(tc.tile_pool(name="work", bufs=2))
            psum = ctx.enter_context(tc.tile_pool(name="psum", bufs=2, space="PSUM"))

            ident = persist.tile([128, 128], BF16)
            make_identity(nc, ident)
            ones_col = persist.tile([128, 1], BF16)
            nc.vector.memset(ones_col, 1.0)
            b_neg10 = persist.tile([128, 1], F32)
            nc.vector.memset(b_neg10, -INV_T)
            # prime the exp table set while the input DMA streams
            prime = persist.tile([128, 1], F32)
            nc.scalar.activation(prime, b_neg10, AF.Exp, bias=b_neg10)

            z_sb = persist.tile([128, NT, D], F32)
            zb = persist.tile([128, NT, D], BF16)
            sq = persist.tile([128, NT, D], BF16)
            ss = persist.tile([128, NT], F32)
            rn = persist.tile([128, NT], F32)
            zn = persist.tile([128, NT, D], BF16)
            znT = persist.tile([128, COLS], BF16)
            ej = persist.tile([128, NBLK, COLS], BF16)
            acc = persist.tile([128, NBLK, len(CHUNKS)], F32)
            cs_sb = persist.tile([1, CS_HI - CS_LO], F32)
            ri = persist.tile([128, NT], I32)
            rt = persist.tile([128, NT], F32)
            ry = persist.tile([128, NT], F32)

            for g in range(NGRP):
                sl = slice(g * GRP, (g + 1) * GRP)
                nc.sync.dma_start(out=z_sb[:, sl, :], in_=z_re[:, sl, :])

            def norm_pre(g):  # DVE: cast, square, row-sumsq
                sl = slice(g * GRP, (g + 1) * GRP)
                nc.vector.tensor_copy(zb[:, sl, :], z_sb[:, sl, :])
                nc.vector.tensor_mul(sq[:, sl, :], zb[:, sl, :], zb[:, sl, :])
                nc.vector.reduce_sum(ss[:, sl], sq[:, sl, :], axis=AX.X)

            def rsqrt(sl):  # DVE bit-trick + 2 Newton: rn[sl] = 1/sqrt(ss[sl])
                s, i, t, y = ss[:, sl], ri[:, sl], rt[:, sl], ry[:, sl]
                nc.vector.tensor_scalar(i, s.bitcast(I32), -1, None, OP.mult)
                nc.vector.tensor_scalar(i, i, 1, None, OP.arith_shift_right)
                nc.vector.tensor_scalar(i, i, MAGIC, None, OP.add)
                y0 = i.bitcast(F32)
                nc.vector.tensor_mul(t, y0, y0)
                nc.vector.tensor_mul(t, s, t)
                nc.vector.tensor_scalar(t, t, -0.5, 1.5, OP.mult, OP.add)
                nc.vector.tensor_mul(y, y0, t)
                nc.vector.tensor_mul(t, y, y)
                nc.vector.tensor_mul(t, s, t)
                nc.vector.tensor_scalar(t, t, -0.5, 1.5, OP.mult, OP.add)
                nc.vector.tensor_mul(rn[:, sl], y, t)

            def norm_fin(g):  # DVE scale + PE transpose into znT
                sl = slice(g * GRP, (g + 1) * GRP)
                nc.vector.tensor_mul(
                    zn[:, sl, :],
                    zb[:, sl, :],
                    rn[:, sl].broadcast_to((128, GRP, D)),
                )
                tp = psum.tile([128, GRP * 128], BF16, tag="pp", name="tp")
                for q in range(GRP):
                    nc.tensor.transpose(
                        tp[:, q * 128 : (q + 1) * 128],
                        zn[:, g * GRP + q, :],
                        ident,
                    )
                nc.vector.tensor_copy(znT[:, g * 1024 : (g + 1) * 1024], tp)

            def emit_chunk(ci, b):
                base, cw = CHUNKS[ci]
                pt = psum.tile([128, 2048], F32, tag="pp", name="pt")
                for s in range(cw // 512):
                    nc.tensor.matmul(
                        pt[:, s * 512 : (s + 1) * 512],
                        znT[:, b * 128 : (b + 1) * 128],
                        znT[:, base + s * 512 : base + (s + 1) * 512],
                        start=True,
                        stop=True,
                    )
                nc.scalar.activation(
                    ej[:, b, base : base + cw],
                    pt[:, :cw],
                    AF.Exp,
                    scale=INV_T,
                    bias=b_neg10,
                    accum_out=acc[:, b, ci : ci + 1],
                )

            def emit_colsum(off):
                cs = psum.tile([1, 512], F32, tag="pp", name="cs")
                for b in range(NBLK):
                    nc.tensor.matmul(
                        cs,
                        ones_col,
                        ej[:, b, off : off + 512],
                        start=(b == 0),
                        stop=(b == NBLK - 1),
                    )
                nc.vector.tensor_copy(cs_sb[:, off - CS_LO : off - CS_LO + 512], cs)

            # groups 0-1 normalize first; chunk-0 exps (cols 0..2047) start
            # while groups 2-4 normalize under the exp stream
            norm_pre(0)
            norm_pre(1)
            rsqrt(slice(0, 2 * GRP))
            norm_fin(0)
            norm_fin(1)
            for b in range(NBLK):
                emit_chunk(0, b)
            norm_pre(2)
            norm_pre(3)
            norm_pre(4)
            rsqrt(slice(2 * GRP, NT))
            norm_fin(2)
            norm_fin(3)
            norm_fin(4)
            for b in range(NBLK):
                emit_chunk(1, b)
            # chunk-0 colsums (cols 1024..2047) run while ACT streams chunk-1
            # exps; chunk-1 colsums interleave with chunk-2 matmul fills.  In
            # both cases the needed ej regions are complete well before the
            # in-order PE queue reaches the colsum matmuls.
            emit_colsum(1024)
            emit_colsum(1536)
            for b in range(NBLK):
                emit_chunk(2, b)
                if b % 2 == 1:
                    emit_colsum(2048 + (b // 2) * 512)

            # ---- epilogue ------------------------------------------------
            # sum_r pos-cosine: zn rows 0..1023 dot zn rows 4096..5119
            pm = work.tile([128, NBLK, D], BF16, tag="pm", name="pm")
            nc.vector.tensor_mul(pm, zn[:, 0:NBLK, :], zn[:, 4 * NBLK : 5 * NBLK, :])
            pr = work.tile([128, NBLK], F32, tag="pr", name="pr")
            nc.vector.reduce_sum(pr, pm, axis=AX.X)
            possum = persist.tile([128, 1], F32)
            nc.vector.reduce_sum(possum, pr, axis=AX.X)
            nc.sync.dma_start(out=out_pos[:, :], in_=possum)

            rowsum = persist.tile([128, NBLK], F32)
            nc.vector.reduce_sum(rowsum, acc, axis=AX.X)
            nc.sync.dma_start(out=out_row[:, :], in_=rowsum)
            nc.sync.dma_start(out=out_cs[:, :], in_=cs_sb)

    nc.compile()
    return nc


_NC = None


def _get_nc() -> bass.Bass:
    global _NC
    if _NC is None:
        _NC = _build()
    return _NC


def make_in_maps(z: np.ndarray) -> list[dict]:
    return [
        {"z": np.ascontiguousarray(np.roll(z, -c * ROWS, axis=0)[:COLS])}
        for c in range(NCORES)
    ]


def kernel(emb0: np.ndarray, emb1: np.ndarray) -> np.ndarray:
    z = np.concatenate(
        [np.asarray(emb0, np.float32), np.asarray(emb1, np.float32)], axis=0
    )
    res = bass_utils.run_bass_kernel_spmd(
        _get_nc(), make_in_maps(z), core_ids=list(range(NCORES))
    )
    # assemble full row sums of exp(10 cos - 10) from per-core partials
    S = np.zeros(N, dtype=np.float64)
    pos_total = 0.0
    for c, r in enumerate(res.results):
        rows = r["rowsum"].astype(np.float64).T.reshape(ROWS)  # local row n*128+p
        S[c * ROWS : (c + 1) * ROWS] += rows
        idx = (c * ROWS + CS_LO + np.arange(CS_HI - CS_LO)) % N
        np.add.at(S, idx, r["colsum"].astype(np.float64).reshape(-1))
        pos_total += float(r["possum"].sum(dtype=np.float64))
    S -= 1.0  # remove the diagonal term exp(10*1 - 10) = 1
    total = float(np.sum(np.log(S))) + INV_T * N - INV_T * pos_total
    return np.asarray(np.float32(total / N))
